# revision 23
# baseline (speedup 1.0000x reference)
"""Trainium2 Bass kernel for nn_Cfc_37546604101675.

Strategy:
  * All SFF pre/post stages are linear in x -> folded on the host into a few
    S x S matrices (parameter-only work, O(S^2) numpy).
  * Device does: x transpose (PE), S-transform matmul, 168-step CfC recurrence
    (the serial bottleneck), and a folded readout.
  * Data parallel over batch: 8 cores x 64 samples. Parameters replicated.
  * Recurrence layout: features on partitions, batch on free dim.
    - phase A: z0 = silu(Wx @ x1_s + (Wh/2) @ h~)          [one ACT op, 128x128]
    - phase B: z1 = silu(Wb1 @ z0)                         [one ACT op, 128x128]
    - phase C: tanh of [ff2|ff1|y/2|-y/2] in one PSUM bank [one ACT op, 128x256]
      (sigmoid folded: t = (1+tanh(y/2))/2)
    - blend:   h~ = (1+tau)*ff2 + (1-tau)*ff1  (= 2*h, the 1/2 folded into
      Wh/2 and Wfc/2), as one scalar_tensor_tensor + one tensor_add on DVE.
    - readout: one tiny matmul per step (lhsT=h~ tile, N=3) accumulating
      R[b, 3s+c] into a single PSUM bank, transposed at the end.
"""

import numpy as np

import concourse.bass as bass
import concourse.tile as tile
from concourse import bacc, mybir
from concourse.bass_utils import run_bass_kernel_spmd
from concourse.masks import make_identity

B, S, F, H, O, U = 512, 168, 12, 128, 3, 256
NCORES = 8
BC = B // NCORES  # 64 samples per core
PERIODS = (3, 6, 12, 24)
FP32 = mybir.dt.float32
AF = mybir.ActivationFunctionType
ALU = mybir.AluOpType

# ---------------------------------------------------------------------------
# Host-side parameter folding (numpy on parameters only)
# ---------------------------------------------------------------------------


def _conv_matrix(conv_w, p):
    k = conv_w.shape[0]
    pad = p // 2
    T = np.eye(S, dtype=np.float64)
    for i in range(S):
        for kk in range(k):
            j = i + kk - pad
            if 0 <= j < S:
                T[i, j] += conv_w[kk]
    return T


def _seg_matrix(lin_w, p):
    seg = S // p
    M = np.zeros((S, S), dtype=np.float64)
    for j in range(p):
        for q in range(seg):
            for k in range(seg):
                M[q * p + j, k * p + j] = lin_w[q, k]
    return M


def _group_matrix(inp, group):
    M = np.zeros((S, S), dtype=np.float64)
    for p in PERIODS:
        cw = np.asarray(inp[f"conv{p}"], np.float64)[group]
        lw = np.asarray(inp[f"lin{p}"], np.float64)[group]
        M += _seg_matrix(lw, p) @ _conv_matrix(cw, p)
    return 0.25 * M


def _fold_params(inp):
    M1 = _group_matrix(inp, 0)
    M2 = _group_matrix(inp, 1)
    M3 = _group_matrix(inp, 2)
    C = np.eye(S) - np.ones((S, S)) / S
    M1c = M1 @ C

    Wb0 = np.asarray(inp["Wb0"], np.float64)
    Wx = Wb0[:, :F]
    Wh2 = Wb0[:, F:] / 2.0
    Wb1 = np.asarray(inp["Wb1"], np.float64)
    Wff1 = np.asarray(inp["Wff1"], np.float64)
    Wff2 = np.asarray(inp["Wff2"], np.float64)
    Wtab2 = 0.5 * (np.asarray(inp["Wta"], np.float64) + np.asarray(inp["Wtb"], np.float64))
    btab2 = 0.5 * (np.asarray(inp["bta"], np.float64) + np.asarray(inp["btb"], np.float64))
    Wfc2 = np.asarray(inp["Wfc"], np.float64) / 2.0
    Wfits = np.asarray(inp["Wfits"], np.float64)
    bfc = np.asarray(inp["bfc"], np.float64)

    # heads stationary: columns [Wff2.T | Wff1.T | Wtab2.T | -Wtab2.T]
    headsT = np.concatenate([Wff2.T, Wff1.T, Wtab2.T, -Wtab2.T], axis=1)  # [256, 512]

    FWT, Hm, bvec = [], [], []
    for c in range(O):
        FW_c = Wfits[c]                       # [p, s]
        K_c = FW_c @ M3 + M2                  # [p, t]
        H_c = (K_c @ M1c).T + np.ones((S, S)) / S
        FWT.append(FW_c.T)
        Hm.append(H_c)
        bvec.append(bfc[c] * FW_c.sum(axis=1))

    # per-partition bias columns for the generic (nonzero bias) path:
    # [bb0c0, bb0c1, bb1c0, bb1c1, bff2, bff1, btab2, -btab2]
    bb0 = np.asarray(inp["bb0"], np.float64)
    bb1 = np.asarray(inp["bb1"], np.float64)
    bias_all = np.stack(
        [
            bb0[:H], bb0[H:], bb1[:H], bb1[H:],
            np.asarray(inp["bff2"], np.float64),
            np.asarray(inp["bff1"], np.float64),
            btab2, -btab2,
        ],
        axis=1,
    )  # [128, 8]

    f32 = lambda a: np.ascontiguousarray(np.asarray(a, np.float32))
    return {
        "m1cT": f32(M1c.T),                 # [t, s]
        "WxT": f32(Wx.T),                   # [12, 256]
        "Wh2T": f32(Wh2.T),                 # [128, 256]
        "Wb1T": f32(Wb1.T),                 # [256, 256]
        "headsT": f32(headsT),              # [256, 512]
        "Wfc2T": f32(Wfc2.T),               # [128, 3]
        "FWT": f32(np.stack(FWT)),          # [3, s, p]
        "Hm": f32(np.stack(Hm)),            # [3, r, p]
        "bias_all": f32(bias_all),          # [128, 8]
        "bvec": f32(np.stack(bvec)),        # [3, p]
        "has_mlp_bias": bool(np.any(bias_all != 0.0)),
        "has_bfc": bool(np.any(bfc != 0.0)),
    }


# ---------------------------------------------------------------------------
# Device kernel
# ---------------------------------------------------------------------------


def _build_bass(has_mlp_bias: bool, has_bfc: bool, taps: bool = False):
    nc = bacc.Bacc("TRN2", target_bir_lowering=False, debug=False)

    x_d = nc.dram_tensor("x", [BC, F, S], FP32, kind="ExternalInput").ap()
    m1cT_d = nc.dram_tensor("m1cT", [S, S], FP32, kind="ExternalInput").ap()
    WxT_d = nc.dram_tensor("WxT", [F, U], FP32, kind="ExternalInput").ap()
    Wh2T_d = nc.dram_tensor("Wh2T", [H, U], FP32, kind="ExternalInput").ap()
    Wb1T_d = nc.dram_tensor("Wb1T", [U, U], FP32, kind="ExternalInput").ap()
    headsT_d = nc.dram_tensor("headsT", [U, 4 * H], FP32, kind="ExternalInput").ap()
    Wfc2T_d = nc.dram_tensor("Wfc2T", [H, O], FP32, kind="ExternalInput").ap()
    FWT_d = nc.dram_tensor("FWT", [O, S, S], FP32, kind="ExternalInput").ap()
    Hm_d = nc.dram_tensor("Hm", [O, S, S], FP32, kind="ExternalInput").ap()
    bias_d = nc.dram_tensor("bias_all", [H, 8], FP32, kind="ExternalInput").ap()
    bvec_d = nc.dram_tensor("bvec", [O, S], FP32, kind="ExternalInput").ap()
    out_d = nc.dram_tensor("out", [BC, S * O], FP32, kind="ExternalOutput").ap()

    TLO = S - H  # 40, the tail of the 168-long axes

    with tile.TileContext(nc) as tc:
        with (
            tc.tile_pool(name="consts", bufs=1) as consts,
            tc.tile_pool(name="bigbuf", bufs=1) as bigbuf,
            tc.tile_pool(name="hsbuf", bufs=1) as hsbuf,
        ):
            # ---- constants / weights to SBUF ----
            ident = consts.tile([128, 128], FP32)
            make_identity(nc, ident)

            m1cT_hi = consts.tile([H, S], FP32)
            m1cT_lo = consts.tile([TLO, S], FP32)
            nc.sync.dma_start(out=m1cT_hi, in_=m1cT_d[:H, :])
            nc.sync.dma_start(out=m1cT_lo, in_=m1cT_d[H:, :])

            WxT_s = consts.tile([F, U], FP32)
            nc.sync.dma_start(out=WxT_s, in_=WxT_d)
            Wh2T_s = consts.tile([H, U], FP32)
            nc.sync.dma_start(out=Wh2T_s, in_=Wh2T_d)
            Wb1T_s0 = consts.tile([H, U], FP32)
            Wb1T_s1 = consts.tile([H, U], FP32)
            nc.sync.dma_start(out=Wb1T_s0, in_=Wb1T_d[:H, :])
            nc.sync.dma_start(out=Wb1T_s1, in_=Wb1T_d[H:, :])
            headsT_s0 = consts.tile([H, 4 * H], FP32)
            headsT_s1 = consts.tile([H, 4 * H], FP32)
            nc.sync.dma_start(out=headsT_s0, in_=headsT_d[:H, :])
            nc.sync.dma_start(out=headsT_s1, in_=headsT_d[H:, :])
            Wfc2T_s = consts.tile([H, O], FP32)
            nc.sync.dma_start(out=Wfc2T_s, in_=Wfc2T_d)

            FWT_hi = consts.tile([H, O, S], FP32)
            FWT_lo = consts.tile([TLO, O, S], FP32)
            Hm_hi = consts.tile([H, O, S], FP32)
            Hm_lo = consts.tile([TLO, O, S], FP32)
            nc.sync.dma_start(out=FWT_hi, in_=FWT_d.rearrange("o s p -> s o p")[:H])
            nc.sync.dma_start(out=FWT_lo, in_=FWT_d.rearrange("o s p -> s o p")[H:])
            nc.sync.dma_start(out=Hm_hi, in_=Hm_d.rearrange("o s p -> s o p")[:H])
            nc.sync.dma_start(out=Hm_lo, in_=Hm_d.rearrange("o s p -> s o p")[H:])

            bias_s = consts.tile([H, 8], FP32)
            nc.sync.dma_start(out=bias_s, in_=bias_d)
            bvec_s = consts.tile([1, O, S], FP32)
            nc.sync.dma_start(out=bvec_s, in_=bvec_d.rearrange("o p -> (o p)")[None, :])
            ones_row = consts.tile([1, BC], FP32)
            nc.vector.memset(ones_row, 1.0)

            # ---- big persistent buffers ----
            xT_hi = bigbuf.tile([H, 768], FP32)     # [t0:128, (c,b) c-major]
            xT_lo = bigbuf.tile([TLO, 768], FP32)   # [t128:168, (c,b)]
            x1T = [bigbuf.tile([128, S], FP32, tag=f"x1T{k}", name=f"x1T{k}") for k in range(6)]
            x1c = bigbuf.tile([F, BC * S], FP32)    # [c, (b, s)]
            hzero = bigbuf.tile([H, BC], FP32)
            nc.vector.memset(hzero, 0.0)
            if taps:
                tap_z0 = bigbuf.tile([H, 2 * BC], FP32)
                tap_z1 = bigbuf.tile([H, 2 * BC], FP32)
                tap_cc = bigbuf.tile([H, 4 * BC], FP32)
            hst = [hsbuf.tile([H, BC], FP32, tag=f"hs{s}", name=f"hs{s}") for s in range(S)]

            # R[b, 3s+c]: per-step readout, copied from a small rotating
            # psum tile into this SBUF accumulator.
            Rsb = bigbuf.tile([BC, S * O], FP32)

            # ---- pre phase: load x as (c b) t (c-major rows), transpose ----
            with (
                tc.tile_pool(name="xin", bufs=3) as xin_pool,
                tc.tile_pool(name="pst", bufs=2, space="PSUM") as pst_pool,
            ):
                x_cb = x_d.rearrange("b c t -> c b t")           # [12, 64, 168]
                for k in range(6):
                    xin = xin_pool.tile([128, S], FP32, tag="xin")
                    for j in range(2):
                        nc.sync.dma_start(
                            out=xin[BC * j : BC * (j + 1), :], in_=x_cb[2 * k + j]
                        )
                    pt_hi = pst_pool.tile([H, 128], FP32, tag="pt_hi")
                    nc.tensor.transpose(pt_hi, xin[:, :H], ident)
                    nc.vector.tensor_copy(xT_hi[:, 128 * k : 128 * (k + 1)], pt_hi)
                    pt_lo = pst_pool.tile([TLO, 128], FP32, tag="pt_lo")
                    nc.tensor.transpose(pt_lo, xin[:, H:], ident)
                    nc.vector.tensor_copy(xT_lo[:, 128 * k : 128 * (k + 1)], pt_lo)

                # stage 1: x1T[k] = (xT chunk k).T @ m1cT
                for k in range(6):
                    ps1 = pst_pool.tile([128, S], FP32, tag="ps1")
                    nc.tensor.matmul(
                        ps1, xT_hi[:, 128 * k : 128 * (k + 1)], m1cT_hi,
                        start=True, stop=False,
                    )
                    nc.tensor.matmul(
                        ps1, xT_lo[:, 128 * k : 128 * (k + 1)], m1cT_lo,
                        start=False, stop=True,
                    )
                    nc.vector.tensor_copy(x1T[k], ps1)

                # remap x1T ((c,b) rows, s cols) -> x1c [c, (b, s)] via DMA
                for k in range(6):
                    for j in range(2):
                        row = 2 * k + j
                        nc.sync.dma_start(
                            out=x1c[row : row + 1, :],
                            in_=x1T[k][BC * j : BC * (j + 1), :],
                        )

            # ---------------- recurrence ----------------
            with (
                tc.tile_pool(name="psA", bufs=2, space="PSUM") as psA_pool,
                tc.tile_pool(name="psB", bufs=2, space="PSUM") as psB_pool,
                tc.tile_pool(name="psC", bufs=2, space="PSUM") as psC_pool,
                tc.tile_pool(name="psR", bufs=2, space="PSUM") as psR_pool,
                tc.tile_pool(name="zbuf", bufs=3) as zbuf,
            ):
                x1c_sb = x1c.rearrange("c (b s) -> c s b", s=S)  # step slice view
                for s in range(S):
                    h_prev = hzero if s == 0 else hst[s - 1]
                    x1_s = x1c_sb[:, s, :]  # [12, 64] (free stride S)

                    psA = psA_pool.tile([H, 2 * BC], FP32, tag="psA")
                    nc.tensor.matmul(psA[:, :BC], WxT_s[:, :H], x1_s, start=True, stop=False)
                    nc.tensor.matmul(psA[:, BC:], WxT_s[:, H:], x1_s, start=False, stop=False)
                    nc.tensor.matmul(psA[:, :BC], Wh2T_s[:, :H], h_prev, start=False, stop=False)
                    nc.tensor.matmul(psA[:, BC:], Wh2T_s[:, H:], h_prev, start=False, stop=True)

                    # previous step's readout matmul goes here so it fills the
                    # PE gap while ACT runs phase A (keeps it off the chain).
                    if s > 0:
                        psr = psR_pool.tile([BC, O], FP32, tag="psr")
                        nc.tensor.matmul(psr, hst[s - 1], Wfc2T_s, start=True, stop=True)
                        nc.vector.tensor_copy(
                            Rsb[:, (s - 1) * O : s * O], psr
                        )

                    z0 = zbuf.tile([H, 2 * BC], FP32, tag="z0")
                    if has_mlp_bias:
                        nc.scalar.activation(z0[:, :BC], psA[:, :BC], AF.Silu, bias=bias_s[:, 0:1])
                        nc.scalar.activation(z0[:, BC:], psA[:, BC:], AF.Silu, bias=bias_s[:, 1:2])
                    else:
                        nc.scalar.activation(z0, psA, AF.Silu)

                    psB = psB_pool.tile([H, 2 * BC], FP32, tag="psB")
                    nc.tensor.matmul(psB[:, :BC], Wb1T_s0[:, :H], z0[:, :BC], start=True, stop=False)
                    nc.tensor.matmul(psB[:, BC:], Wb1T_s0[:, H:], z0[:, :BC], start=False, stop=False)
                    nc.tensor.matmul(psB[:, :BC], Wb1T_s1[:, :H], z0[:, BC:], start=False, stop=False)
                    nc.tensor.matmul(psB[:, BC:], Wb1T_s1[:, H:], z0[:, BC:], start=False, stop=True)

                    z1 = zbuf.tile([H, 2 * BC], FP32, tag="z1")
                    if has_mlp_bias:
                        nc.scalar.activation(z1[:, :BC], psB[:, :BC], AF.Silu, bias=bias_s[:, 2:3])
                        nc.scalar.activation(z1[:, BC:], psB[:, BC:], AF.Silu, bias=bias_s[:, 3:4])
                    else:
                        nc.scalar.activation(z1, psB, AF.Silu)

                    psC = psC_pool.tile([H, 4 * BC], FP32, tag="psC")
                    for m in range(4):
                        nc.tensor.matmul(
                            psC[:, m * BC : (m + 1) * BC],
                            headsT_s0[:, m * H : (m + 1) * H], z1[:, :BC],
                            start=(m == 0), stop=False,
                        )
                        nc.tensor.matmul(
                            psC[:, m * BC : (m + 1) * BC],
                            headsT_s1[:, m * H : (m + 1) * H], z1[:, BC:],
                            start=False, stop=(m == 3),
                        )

                    cc = zbuf.tile([H, 4 * BC], FP32, tag="cc")
                    if has_mlp_bias:
                        for m in range(4):
                            nc.scalar.activation(
                                cc[:, m * BC : (m + 1) * BC],
                                psC[:, m * BC : (m + 1) * BC],
                                AF.Tanh, bias=bias_s[:, 4 + m : 5 + m],
                            )
                    else:
                        nc.scalar.activation(cc, psC, AF.Tanh)

                    if taps and s == 0:
                        nc.vector.tensor_copy(tap_z0, z0)
                        nc.vector.tensor_copy(tap_z1, z1)
                        nc.vector.tensor_copy(tap_cc, cc)

                    # blend: P = ([tau|-tau] + 1) * [ff2|ff1]; h~ = P_l + P_r
                    P = zbuf.tile([H, 2 * BC], FP32, tag="P")
                    nc.vector.scalar_tensor_tensor(
                        P, cc[:, 2 * BC :], 1.0, cc[:, : 2 * BC],
                        op0=ALU.add, op1=ALU.mult,
                    )
                    nc.vector.tensor_add(hst[s], P[:, :BC], P[:, BC:])

                # last step's readout
                psr = psR_pool.tile([BC, O], FP32, tag="psr")
                nc.tensor.matmul(psr, hst[S - 1], Wfc2T_s, start=True, stop=True)
                nc.vector.tensor_copy(Rsb[:, (S - 1) * O :], psr)

            # ---------------- post phase ----------------
            with (
                tc.tile_pool(name="psT", bufs=1, space="PSUM") as psT_pool,
                tc.tile_pool(name="psO", bufs=2, space="PSUM") as psO_pool,
                tc.tile_pool(name="postbuf", bufs=1) as postbuf,
            ):
                # R: [64, (s,o)] -> per-channel R_c [64, 168] is a stride-O slice
                Rv = Rsb.rearrange("b (s o) -> b s o", o=O)

                outbuf = postbuf.tile([BC, S * O], FP32)
                ob_v = outbuf.rearrange("b (s o) -> b s o", o=O)
                for c in range(O):
                    # transpose R_c [64, 168] -> RT_c ([128,64] + [40,64])
                    ptr_hi = psT_pool.tile([H, BC], FP32, tag="ptr_hi")
                    nc.tensor.transpose(ptr_hi, Rv[:, :H, c], ident[:BC, :BC])
                    RT_hi = postbuf.tile([H, BC], FP32, tag=f"RT_hi{c}", name=f"RT_hi{c}")
                    nc.vector.tensor_copy(RT_hi, ptr_hi)
                    ptr_lo = psT_pool.tile([TLO, BC], FP32, tag="ptr_lo")
                    nc.tensor.transpose(ptr_lo, Rv[:, H:, c], ident[:BC, :BC])
                    RT_lo = postbuf.tile([TLO, BC], FP32, tag=f"RT_lo{c}", name=f"RT_lo{c}")
                    nc.vector.tensor_copy(RT_lo, ptr_lo)

                    psO = psO_pool.tile([BC, S], FP32, tag="psO")
                    nc.tensor.matmul(psO, RT_hi, FWT_hi[:, c, :], start=True, stop=False)
                    nc.tensor.matmul(psO, RT_lo, FWT_lo[:, c, :], start=False, stop=False)
                    nc.tensor.matmul(
                        psO, xT_hi[:, c * BC : (c + 1) * BC], Hm_hi[:, c, :],
                        start=False, stop=False,
                    )
                    nc.tensor.matmul(
                        psO, xT_lo[:, c * BC : (c + 1) * BC], Hm_lo[:, c, :],
                        start=False, stop=not has_bfc,
                    )
                    if has_bfc:
                        nc.tensor.matmul(
                            psO, ones_row, bvec_s[:, c, :], start=False, stop=True
                        )
                    nc.vector.tensor_copy(ob_v[:, :, c], psO)
                nc.sync.dma_start(out=out_d, in_=outbuf)

                if taps:
                    tap_specs = {
                        "tap_ident": ident,
                        "tap_xT_hi": xT_hi,
                        "tap_xT_lo": xT_lo,
                        "tap_x1T0": x1T[0],
                        "tap_x1c": x1c,
                        "tap_z0": tap_z0,
                        "tap_z1": tap_z1,
                        "tap_cc": tap_cc,
                        "tap_hs0": hst[0],
                        "tap_hs1": hst[1],
                        "tap_hs167": hst[S - 1],
                        "tap_R": Rsb,
                    }
                    for nm, t in tap_specs.items():
                        td = nc.dram_tensor(nm, list(t.shape), FP32, kind="ExternalOutput").ap()
                        nc.sync.dma_start(out=td, in_=t)

    nc.compile()
    return nc


_BUILD_CACHE = {}


def kernel(**inputs):
    x = np.ascontiguousarray(np.asarray(inputs["x"], np.float32))  # [B, 1, F, S]
    fp = _fold_params(inputs)

    key = (fp["has_mlp_bias"], fp["has_bfc"])
    if key not in _BUILD_CACHE:
        _BUILD_CACHE[key] = _build_bass(*key)
    nc = _BUILD_CACHE[key]

    params = {
        k: fp[k]
        for k in ("m1cT", "WxT", "Wh2T", "Wb1T", "headsT", "Wfc2T", "FWT", "Hm",
                  "bias_all", "bvec")
    }
    in_maps = []
    for i in range(NCORES):
        m = dict(params)
        m["x"] = np.ascontiguousarray(x[i * BC : (i + 1) * BC, 0])  # [64, 12, 168]
        in_maps.append(m)

    res = run_bass_kernel_spmd(nc, in_maps, list(range(NCORES)))
    out = np.concatenate(
        [r["out"].reshape(BC, S, O) for r in res.results], axis=0
    )
    return out.astype(np.float32)
